# revision 32
# baseline (speedup 1.0000x reference)
"""Talking-heads attention on 8 Trainium2 NeuronCores.

Sharding: data-parallel over (batch b in 0..3) x (query half in 0..1) -> 8 cores.
Each core computes K/V for its full batch sequence (1024) and attention for its
512 query rows. No collectives.

Algorithm (per core) — per-head dots with packed kron head-mixes:
  - qT[hd, i], kT[hd, j], V[j, (k,d)] projections (SCALE folded into Wq host-side).
  - dots per head h: psum[j128, i-cols] = kT_h[d, jblk]^T qT_h[d, i]  (64-contraction).
  - drain interleaves dots into X[j, jc, ig, (i10, h12)] bf16 (i padded 512->520,
    52 groups of 10 i-slots x 12 heads = 120 columns per group).
  - T1: PE-transpose each [j128, 120] group -> [(i10,h12), j128] (bf16 PSUM).
  - mix_pre: lhsT = kron(I10, M1) [120,120] (host-built), rhs = dotsT[(i,h), j]
    -> mixed dots [(i,g), j] in PSUM.
  - exp on ACT with accum_out -> attnE[(i,g), j] bf16 + rowsum S[(i,g)].
  - mix_post + transpose-back fused in one matmul: lhsT = attnE[(i,h), jblk]
    (m=j128), rhs = kron(I10, M2) * (1/S) broadcast per partition ->
    out[j128, (i,k)] = attn-mixed-normalized, transposed for AV.
  - AV: out2[(k,d), i] += V[j, (k,d)]^T attnT[j, (i at stride 12, k)] over jc.
  - out = out2^T @ Wout + bout.
"""

import numpy as np

import concourse.bass as bass
import concourse.mybir as mybir
import concourse.tile as tile
from concourse import bacc
from concourse.bass_utils import run_bass_kernel_spmd

P = 128
DIM = 768
SEQ = 1024
IQ = 512            # query rows per core
H = 12
DH = 64
NC6 = DIM // P      # 6 chunks of the 768 dim
JC8 = SEQ // P      # 8 chunks of the key dim
ISUB = 10           # i-slots per packed group
G = 120             # (ISUB, H) packed partitions
NGH = 26            # i-groups per round (2 rounds of 26 -> 52 groups, 520 slots)
SCALE = DH ** -0.5
F32 = mybir.dt.float32
BF16 = mybir.dt.bfloat16

_CACHE = {}


def _build_nc():
    nc = bacc.Bacc("TRN2", target_bir_lowering=False, debug=False)

    xqT = nc.dram_tensor("xqT", [DIM, IQ], BF16, kind="ExternalInput")
    xkvT = nc.dram_tensor("xkvT", [DIM, SEQ], BF16, kind="ExternalInput")
    Wq = nc.dram_tensor("Wq", [DIM, DIM], BF16, kind="ExternalInput")
    Wk = nc.dram_tensor("Wk", [DIM, DIM], BF16, kind="ExternalInput")
    Wv = nc.dram_tensor("Wv", [DIM, DIM], BF16, kind="ExternalInput")
    Wout = nc.dram_tensor("Wout", [DIM, DIM], BF16, kind="ExternalInput")
    bout = nc.dram_tensor("bout", [1, DIM], F32, kind="ExternalInput")
    kron1 = nc.dram_tensor("kron1", [G, G], BF16, kind="ExternalInput")
    kron2b = nc.dram_tensor("kron2b", [G, G], F32, kind="ExternalInput")
    out = nc.dram_tensor("out", [IQ, DIM], F32, kind="ExternalOutput")

    r3 = lambda t: t.rearrange("(c p) e -> p c e", p=P)

    with tile.TileContext(nc) as tc:
        with tc.tile_pool(name="persist", bufs=1) as pp:
            # ---- persistent tiles ----
            qT = pp.tile([P, NC6, 520], BF16)  # i padded 512->520 (pad memset 0)
            kT = pp.tile([P, NC6, SEQ], BF16)
            V = pp.tile([P, JC8, DIM], BF16)      # [j-part, jc, (k,d)]
            Wout_sb = pp.tile([P, NC6, DIM], BF16)
            bout_t = pp.tile([P, DIM], F32)
            o2_sb = pp.tile([P, NC6, IQ], BF16)   # [(kpair,d), kdchunk, i]
            kron1_sb = pp.tile([G, G], BF16)
            kron2b_sb = pp.tile([G, G], F32)
            ident = pp.tile([P, P], BF16)
            # X: dots-interleaved [j, jc, ig, h12, i10]; reused as attnT
            # [j, jb, ig, k12, i10] after the fused mix2-transpose.
            X = pp.tile([P, JC8, NGH, H, ISUB], BF16)
            # Y: dotsT [(i,h), ig, j]; exp overwrites same slices -> attnE.
            Y = pp.tile([G, NGH, SEQ], BF16)

            nc.gpsimd.dma_start(kron1_sb[:], kron1[:])
            nc.gpsimd.dma_start(kron2b_sb[:], kron2b[:])
            nc.gpsimd.dma_start(Wout_sb[:], r3(Wout))

            # identity for PE transposes
            nc.gpsimd.memset(ident[:], 1.0)
            nc.gpsimd.affine_select(
                out=ident[:], in_=ident[:], compare_op=mybir.AluOpType.is_ge,
                fill=0.0, base=0, pattern=[[1, P]], channel_multiplier=-1,
            )
            nc.gpsimd.affine_select(
                out=ident[:], in_=ident[:], compare_op=mybir.AluOpType.is_ge,
                fill=0.0, base=0, pattern=[[-1, P]], channel_multiplier=1,
            )

            # ---- phase 1: projections ----
            with (
                tc.tile_pool(name="pin", bufs=1) as pin,
                tc.tile_pool(name="pj", bufs=2, space="PSUM") as pj,
                tc.tile_pool(name="pjv", bufs=2, space="PSUM") as pjv,
            ):
                xqT_sb = pin.tile([P, NC6, IQ], BF16)
                xkvT_sb = pin.tile([P, NC6, SEQ], BF16)
                Wq_sb = pin.tile([P, NC6, DIM], BF16)
                Wk_sb = pin.tile([P, NC6, DIM], BF16)
                Wv_sb = pin.tile([P, NC6, DIM], BF16)
                bout_sb = pin.tile([1, DIM], F32)
                ones1 = pin.tile([1, P], F32)
                nc.gpsimd.memset(ones1[:], 1.0)
                nc.gpsimd.dma_start(Wq_sb[:], r3(Wq))
                nc.gpsimd.dma_start(xqT_sb[:], r3(xqT))
                nc.gpsimd.dma_start(xkvT_sb[:], r3(xkvT))
                nc.gpsimd.dma_start(Wk_sb[:], r3(Wk))
                nc.gpsimd.dma_start(Wv_sb[:], r3(Wv))
                nc.gpsimd.dma_start(bout_sb[:], bout[:])

                # qT[e,i] = sum_f Wq[f,e] xqT[f,i]   (SCALE folded host-side)
                nc.gpsimd.memset(qT[:, :, IQ:520], 0.0)
                for ec in range(NC6):
                    ps = pj.tile([P, IQ], F32, tag="pjq")
                    for fc in range(NC6):
                        nc.tensor.matmul(
                            ps[:], Wq_sb[:, fc, ec * P : (ec + 1) * P],
                            xqT_sb[:, fc, :], start=(fc == 0), stop=(fc == NC6 - 1),
                        )
                    if ec % 2 == 0:
                        nc.vector.tensor_copy(qT[:, ec, :IQ], ps[:])
                    else:
                        nc.scalar.copy(qT[:, ec, :IQ], ps[:])

                # kT[e,j]
                for ec in range(NC6):
                    for jh in range(2):
                        ps = pj.tile([P, IQ], F32, tag="pjq")
                        for fc in range(NC6):
                            nc.tensor.matmul(
                                ps[:], Wk_sb[:, fc, ec * P : (ec + 1) * P],
                                xkvT_sb[:, fc, jh * IQ : (jh + 1) * IQ],
                                start=(fc == 0), stop=(fc == NC6 - 1),
                            )
                        if (ec * 2 + jh) % 2 == 0:
                            nc.vector.tensor_copy(
                                kT[:, ec, jh * IQ : (jh + 1) * IQ], ps[:]
                            )
                        else:
                            nc.scalar.copy(
                                kT[:, ec, jh * IQ : (jh + 1) * IQ], ps[:]
                            )

                # V[j, (k,d)] = sum_f xkvT[f, j] Wv[f, kd]
                for jc in range(JC8):
                    ps = pjv.tile([P, DIM], F32, tag="pjv")
                    for ns, ne in ((0, 512), (512, DIM)):
                        for fc in range(NC6):
                            nc.tensor.matmul(
                                ps[:, ns:ne],
                                xkvT_sb[:, fc, jc * P : (jc + 1) * P],
                                Wv_sb[:, fc, ns:ne],
                                start=(fc == 0), stop=(fc == NC6 - 1),
                            )
                    if jc % 2 == 0:
                        nc.vector.tensor_copy(V[:, jc, :], ps[:])
                    else:
                        nc.scalar.copy(V[:, jc, :], ps[:])

                # bout broadcast to [128, DIM]
                psb = pjv.tile([P, DIM], F32, tag="pjv")
                for ns, ne in ((0, 512), (512, DIM)):
                    nc.tensor.matmul(
                        psb[:, ns:ne], ones1[:], bout_sb[:, ns:ne],
                        start=True, stop=True,
                    )
                nc.vector.tensor_copy(bout_t[:], psb[:])

            # ---- phase 2: attention ----
            with (
                tc.tile_pool(name="pdot", bufs=3, space="PSUM") as pdot,
                tc.tile_pool(name="pt1", bufs=1, space="PSUM") as pt1,
                tc.tile_pool(name="pmix", bufs=2, space="PSUM") as pmix,
                tc.tile_pool(name="pcb", bufs=2, space="PSUM") as pcb,
                tc.tile_pool(name="sp", bufs=3) as sp,
                tc.tile_pool(name="kp", bufs=4) as kp,
                tc.tile_pool(name="oBuf", bufs=2) as ob,
            ):
                drain_i = [0]

                def drain(dst, src):
                    # f32 psum drains alternate DVE/ACT
                    drain_i[0] += 1
                    if drain_i[0] % 2 != 0:
                        nc.vector.tensor_copy(dst, src)
                    else:
                        nc.scalar.copy(dst, src)

                def emit_chainB(igl, k2):
                    # mix_post fused with transpose-back:
                    # out[j128, (k,i)] = attnE[:, jblk]^T @ (kron2/S)
                    for half in range(2):
                        cb = pcb.tile([P, 4, G], F32, tag="cb")
                        for jbl in range(4):
                            jb = half * 4 + jbl
                            nc.tensor.matmul(
                                cb[:, jbl, :],
                                Y[:, igl, jb * P : (jb + 1) * P],
                                k2[:],
                                start=True, stop=True,
                            )
                        drain(X[:, half * 4 : half * 4 + 4, igl, :, :], cb[:])

                def emit_T1(igl):
                    # T1: transpose [j128, 120] -> [(h,i)120, j128]
                    t1 = pt1.tile([G, JC8, P], BF16, tag="t1", name="t1")
                    for jc in range(JC8):
                        nc.tensor.transpose(
                            t1[:, jc, :], X[:, jc, igl, :, :], ident[:]
                        )
                    # drain in halves so mix1-jh0 only waits on the first
                    nc.vector.tensor_copy(Y[:, igl, 0:IQ], t1[:, 0:4, :])
                    nc.vector.tensor_copy(Y[:, igl, IQ:SEQ], t1[:, 4:8, :])

                def emit_mix_exp(igl):
                    # mix_pre: [(g,i), j] = kron1^T @ dotsT, then exp + rowsum
                    S_halves = []
                    for jh in range(2):
                        mx = pmix.tile([G, IQ], F32, tag="mx", name="mx")
                        nc.tensor.matmul(
                            mx[:], kron1_sb[:],
                            Y[:, igl, jh * IQ : (jh + 1) * IQ],
                            start=True, stop=True,
                        )
                        Sh = sp.tile([G, 1], F32, tag=f"sh{jh}", name=f"Sh{jh}")
                        nc.scalar.activation(
                            Y[:, igl, jh * IQ : (jh + 1) * IQ], mx[:],
                            mybir.ActivationFunctionType.Exp,
                            accum_out=Sh[:],
                        )
                        S_halves.append(Sh)
                    S = sp.tile([G, 1], F32, tag="s", name="S")
                    nc.vector.tensor_tensor(
                        S[:], S_halves[0][:], S_halves[1][:],
                        mybir.AluOpType.add,
                    )
                    rS = sp.tile([G, 1], F32, tag="rs", name="rS")
                    rscr = sp.tile([G, 1], F32, tag="rscr", name="rscr")
                    nc.vector.reciprocal_approx_accurate(
                        out=rS[:], in_=S[:], scratch=rscr[:]
                    )
                    k2 = kp.tile([G, G], BF16, tag="k2", name="k2")
                    nc.vector.tensor_scalar_mul(k2[:], kron2b_sb[:], rS[:])
                    return k2

                for r, i0 in enumerate((0, 260)):
                    # --- dots for this i-half, all heads ---
                    for h in range(H):
                        hb = (h % 2) * DH
                        hc = h // 2
                        for jc in range(JC8):
                            ps = pdot.tile([P, 260], F32, tag="dot", name="dps")
                            nc.tensor.matmul(
                                ps[:],
                                kT[hb : hb + DH, hc, jc * P : (jc + 1) * P],
                                qT[hb : hb + DH, hc, i0 : i0 + 260],
                                start=True, stop=True,
                            )
                            drain(X[:, jc, :, h, :], ps[:])

                    # --- mid pipeline, software-pipelined by 2 groups:
                    # mix1/exp at ig-1, chainB at ig-2, then T1 at ig (last,
                    # so the single t1 psum buffer has a full iteration of
                    # PE work to cover its drain latency) ---
                    k2s = {}
                    for igl in range(NGH):
                        if igl >= 1:
                            k2s[igl - 1] = emit_mix_exp(igl - 1)
                        if igl >= 2:
                            emit_chainB(igl - 2, k2s.pop(igl - 2))
                        emit_T1(igl)
                    k2s[NGH - 1] = emit_mix_exp(NGH - 1)
                    emit_chainB(NGH - 2, k2s.pop(NGH - 2))
                    emit_chainB(NGH - 1, k2s.pop(NGH - 1))

                    # --- AV for this chunk ---
                    ncols = 260 if r == 0 else IQ - 260
                    for kpi in range(NC6):
                        av = pdot.tile([P, 260], F32, tag="dot", name="av")
                        for kl in range(2):
                            k = 2 * kpi + kl
                            for jc in range(JC8):
                                nc.tensor.matmul(
                                    av[kl * DH : (kl + 1) * DH, :],
                                    V[:, jc, k * DH : (k + 1) * DH],
                                    X[:, jc, :, k, :],
                                    start=(jc == 0), stop=(jc == JC8 - 1),
                                )
                        drain(o2_sb[:, kpi, i0 : i0 + ncols], av[:, :ncols])

                    # --- output projection for the i-blocks completed ---
                    for isl in range(2 * r, 2 * r + 2):
                        osb = ob.tile([P, DIM], F32, tag="osb", name="osb")
                        for nchunk in range(3):
                            ns = nchunk * 256
                            fp = pdot.tile([P, 256], F32, tag="dot", name="fp")
                            for c in range(NC6):
                                nc.tensor.matmul(
                                    fp[:],
                                    o2_sb[:, c, isl * P : (isl + 1) * P],
                                    Wout_sb[:, c, ns : ns + 256],
                                    start=(c == 0), stop=(c == NC6 - 1),
                                )
                            nc.vector.tensor_tensor(
                                osb[:, ns : ns + 256], fp[:],
                                bout_t[:, ns : ns + 256], mybir.AluOpType.add,
                            )
                        nc.gpsimd.dma_start(out[isl * P : (isl + 1) * P, :], osb[:])

    nc.compile()
    return nc


def _prep_in_maps(x, Wq, Wkv, mix_pre, mix_post, Wout, bout):
    import ml_dtypes
    bf = ml_dtypes.bfloat16
    Wk = np.ascontiguousarray(Wkv[:, :DIM]).astype(bf)
    Wv = np.ascontiguousarray(Wkv[:, DIM:]).astype(bf)
    kron1 = np.kron(mix_pre, np.eye(ISUB, dtype=np.float32)).astype(bf)
    kron2b = np.ascontiguousarray(
        np.kron(mix_post, np.eye(ISUB, dtype=np.float32))
    )
    shared = {
        "Wq": (Wq * SCALE).astype(bf), "Wk": Wk, "Wv": Wv,
        "Wout": Wout.astype(bf),
        "bout": np.ascontiguousarray(bout.reshape(1, DIM)).astype(np.float32),
        "kron1": kron1, "kron2b": kron2b,
    }
    in_maps = []
    for c in range(8):
        b, half = c // 2, c % 2
        m = dict(shared)
        m["xqT"] = np.ascontiguousarray(
            x[b, half * IQ : (half + 1) * IQ, :].T
        ).astype(bf)
        m["xkvT"] = np.ascontiguousarray(x[b].T).astype(bf)
        in_maps.append(m)
    return in_maps


def kernel(x, Wq, Wkv, mix_pre, mix_post, Wout, bout):
    x = np.asarray(x, dtype=np.float32)
    Wq = np.asarray(Wq, dtype=np.float32)
    Wkv = np.asarray(Wkv, dtype=np.float32)
    mix_pre = np.asarray(mix_pre, dtype=np.float32)
    mix_post = np.asarray(mix_post, dtype=np.float32)
    Wout = np.asarray(Wout, dtype=np.float32)
    bout = np.asarray(bout, dtype=np.float32)

    if "nc" not in _CACHE:
        _CACHE["nc"] = _build_nc()
    nc = _CACHE["nc"]

    in_maps = _prep_in_maps(x, Wq, Wkv, mix_pre, mix_post, Wout, bout)
    res = run_bass_kernel_spmd(nc, in_maps, core_ids=list(range(8)))
    _CACHE["last_results"] = res

    b_, n_, d_ = x.shape
    full = np.empty((b_, n_, d_), dtype=np.float32)
    for c in range(8):
        b, half = c // 2, c % 2
        full[b, half * IQ : (half + 1) * IQ, :] = res.results[c]["out"]
    return full
